# revision 5
# baseline (speedup 1.0000x reference)
"""Multi-head attention TRN2 Bass kernel, sharded over 8 NeuronCores.

Sharding: core c -> (batch b = c//4, head-group g = c%4).  Each core computes
4 heads' worth of Q/K/V projections + attention for one batch element, plus
the partial output projection for its 256-column slice of the head-concat
dimension.  Host sums the 4 partials per batch and adds bf.

Device-side layout tricks:
  - Host pre-transposes x to [DIM, S] so the contraction dim is on partitions.
  - Scores are computed transposed (S^T[kt, qt]) so softmax masking/sums work
    via matmul; head pairs use disjoint PE row halves (K=64 each) concurrently.
  - pad_mask is folded into V and into an extra ones-column of V, so the AV
    matmul produces both the masked numerator and the softmax denominator,
    and exp needs no mask bias at all (exp of raw scores is safe: |s|<~1).
"""

import os
import numpy as np
import ml_dtypes

B, S, DIM, H, DH = 2, 2048, 1024, 16, 64
NCORES = 8
HPC = 4           # heads per core
CSL = HPC * DH    # 256: per-core slice of the head-concat dim
P = 128
KO = DIM // P     # 8 contraction chunks for projections
CC = CSL // P     # 2 col chunks (2 head-pairs)
NKT = S // P      # 16 key-token chunks
QT = 512          # query tile (free dim)
NQT = S // QT     # 4 query tiles
TOK = S // P      # 16 token chunks

BF16 = ml_dtypes.bfloat16

_CACHE = {}
LAST_RESULTS = None


def _build():
    import concourse.bass as bass
    import concourse.tile as tile
    from concourse import bacc, mybir
    from concourse.bass import ts

    f32 = mybir.dt.float32
    bf16 = mybir.dt.bfloat16

    nc = bacc.Bacc("TRN2", target_bir_lowering=False, debug=False)

    xq = nc.dram_tensor("xq", [DIM, S], bf16, kind="ExternalInput").ap()
    xk = nc.dram_tensor("xk", [DIM, S], bf16, kind="ExternalInput").ap()
    xv = nc.dram_tensor("xv", [DIM, S], bf16, kind="ExternalInput").ap()
    wq = nc.dram_tensor("wq", [DIM, CSL], bf16, kind="ExternalInput").ap()
    wk = nc.dram_tensor("wk", [DIM, CSL], bf16, kind="ExternalInput").ap()
    wv = nc.dram_tensor("wv", [DIM, CSL], bf16, kind="ExternalInput").ap()
    wf = nc.dram_tensor("wf", [CSL, DIM], bf16, kind="ExternalInput").ap()
    bq = nc.dram_tensor("bq", [CSL], f32, kind="ExternalInput").ap()
    bk = nc.dram_tensor("bk", [CSL], f32, kind="ExternalInput").ap()
    bv = nc.dram_tensor("bv", [CSL], f32, kind="ExternalInput").ap()
    m01 = nc.dram_tensor("m01", [S], f32, kind="ExternalInput").ap()
    y = nc.dram_tensor("y", [S, DIM], f32, kind="ExternalOutput").ap()

    Exp = mybir.ActivationFunctionType.Exp
    MUL = mybir.AluOpType.mult

    with tile.TileContext(nc) as tc:
        with (
            tc.tile_pool(name="const", bufs=1) as const,
            tc.tile_pool(name="xql", bufs=3) as xql_pool,
            tc.tile_pool(name="xvl", bufs=3) as xvl_pool,
            tc.tile_pool(name="qkv", bufs=1) as qkv,
            tc.tile_pool(name="es", bufs=2) as es_pool,
            tc.tile_pool(name="ot", bufs=2) as ot_pool,
            tc.tile_pool(name="ysb", bufs=3) as ysb_pool,
            tc.tile_pool(name="rc", bufs=2) as rc_pool,
            tc.tile_pool(name="dscr", bufs=2, space="DRAM") as dram_pool,
            tc.tile_pool(name="stp", bufs=1, space="PSUM") as st_psum,
            tc.tile_pool(name="avp", bufs=2, space="PSUM") as av_psum,
            tc.tile_pool(name="mmp", bufs=2, space="PSUM") as mm_psum,
        ):
            # ---- constants ----
            wq_sb = const.tile([P, KO, CSL], bf16)
            wk_sb = const.tile([P, KO, CSL], bf16)
            wv_sb = const.tile([P, KO, CSL], bf16)
            nc.sync.dma_start(wk_sb, wk.rearrange("(ko p) e -> p ko e", p=P))
            nc.sync.dma_start(wv_sb, wv.rearrange("(ko p) e -> p ko e", p=P))
            nc.sync.dma_start(wq_sb, wq.rearrange("(ko p) e -> p ko e", p=P))
            wf_sb = const.tile([P, CC, DIM], bf16)
            nc.sync.dma_start(wf_sb, wf.rearrange("(cc p) e -> p cc e", p=P))
            bq_sb = const.tile([P, CC], f32)
            bk_sb = const.tile([P, CC], f32)
            nc.sync.dma_start(bk_sb, bk.rearrange("(cc p) -> p cc", p=P))
            nc.sync.dma_start(bq_sb, bq.rearrange("(cc p) -> p cc", p=P))
            # bv replicated to all partitions (DMA partition-broadcast)
            bv_sb = const.tile([P, CSL], f32)
            nc.sync.dma_start(bv_sb, bv[None, :].to_broadcast((P, CSL)))
            m01_sb = const.tile([P, NKT], f32)
            nc.sync.dma_start(m01_sb, m01.rearrange("(c p) -> p c", p=P))

            xq_r = xq.rearrange("(ko p) s -> p ko s", p=P)
            xk_r = xk.rearrange("(ko p) s -> p ko s", p=P)
            xv_r = xv.rearrange("(ko p) s -> p ko s", p=P)
            y_r = y.rearrange("(t p) e -> t p e", p=P)

            # ---- K^T projection: kt_sb[p, cc, s] = (key @ Wk + bk)^T ----
            qt_sb = qkv.tile([P, CC, S], bf16)
            kt_sb = qkv.tile([P, CC, S], bf16)
            for t in range(NQT):
                xt = xql_pool.tile([P, KO, QT], bf16, tag="xql")
                nc.sync.dma_start(xt, xk_r[:, :, ts(t, QT)])
                for cc in range(CC):
                    ps = mm_psum.tile([P, QT], f32, tag="mmp")
                    for ko in range(KO):
                        nc.tensor.matmul(
                            ps, lhsT=wk_sb[:, ko, ts(cc, P)], rhs=xt[:, ko, :],
                            start=(ko == 0), stop=(ko == KO - 1),
                        )
                    nc.vector.tensor_add(
                        out=kt_sb[:, cc, ts(t, QT)], in0=ps,
                        in1=bk_sb[:, cc, None].to_broadcast((P, QT)),
                    )

            # ---- V projection, masked + augmented ----
            # Per head h, vaug[:, kc, h, :] is the 128-wide AV stationary
            # operand: even h -> [V(64) | m01 | 0...], odd h -> [m01 | 0(63) | V(64)].
            # So AV psum rows are: even: O at 0..63, denom at 64;
            #                      odd:  denom at 0, O at 64..127.
            vaug = qkv.tile([P, NKT, HPC, P], bf16)
            nc.vector.memset(vaug, 0.0)
            # h = hp*2 + par
            vaug_v = vaug.rearrange("p c (hp par) w -> p par c hp w", par=2)
            nc.vector.tensor_copy(
                out=vaug_v[:, 0, :, :, DH],
                in_=m01_sb[:, :, None].to_broadcast((P, NKT, 2)),
            )
            nc.vector.tensor_copy(
                out=vaug_v[:, 1, :, :, 0],
                in_=m01_sb[:, :, None].to_broadcast((P, NKT, 2)),
            )
            bv_v = bv_sb.rearrange("p (hp par d) -> p par hp d", par=2, d=DH)
            for t in range(TOK):
                xt = xvl_pool.tile([P, KO, P], bf16, tag="xvl")
                nc.sync.dma_start(xt, xv_r[:, :, ts(t, P)])
                ps = mm_psum.tile([P, CSL], f32, tag="mmp")
                for ko in range(KO):
                    nc.tensor.matmul(
                        ps, lhsT=xt[:, ko, :], rhs=wv_sb[:, ko, :],
                        start=(ko == 0), stop=(ko == KO - 1),
                    )
                ps_v = ps.rearrange("p (hp par d) -> p par hp d", par=2, d=DH)
                for par, dlo in ((0, 0), (1, DH)):
                    dst = vaug_v[:, par, t, :, dlo:dlo + DH]
                    nc.vector.tensor_add(
                        out=dst, in0=ps_v[:, par, :, :], in1=bv_v[:, par, :, :],
                    )
                    nc.vector.tensor_tensor(
                        out=dst, in0=dst,
                        in1=m01_sb[:, t, None, None].to_broadcast((P, 2, DH)),
                        op=MUL,
                    )

            # ---- Q^T projection (per q-tile, interleaved with attention) ----
            for t in range(NQT):
                xt = xql_pool.tile([P, KO, QT], bf16, tag="xql")
                nc.sync.dma_start(xt, xq_r[:, :, ts(t, QT)])
                for cc in range(CC):
                    ps = mm_psum.tile([P, QT], f32, tag="mmp")
                    for ko in range(KO):
                        nc.tensor.matmul(
                            ps, lhsT=wq_sb[:, ko, ts(cc, P)], rhs=xt[:, ko, :],
                            start=(ko == 0), stop=(ko == KO - 1),
                        )
                    nc.vector.tensor_add(
                        out=qt_sb[:, cc, ts(t, QT)], in0=ps,
                        in1=bq_sb[:, cc, None].to_broadcast((P, QT)),
                    )

            # ---- attention + output projection, per q-tile ----
            for t in range(NQT):
                ot = ot_pool.tile([P, CC, QT], bf16, tag="ot")
                for j in range(CC):  # head pair j -> heads (2j, 2j+1)
                    es = es_pool.tile([P, NKT, 2, QT], bf16, tag="es")
                    for g in range(NKT // 2):
                        stp = st_psum.tile([P, 2, 2, QT], f32, tag="stp")
                        for i in range(2):
                            kc = 2 * g + i
                            # head 2j on PE rows 0-63, head 2j+1 on rows 64-127
                            nc.tensor.matmul(
                                stp[:, i, 0, :],
                                lhsT=kt_sb[0:DH, j, ts(kc, P)],
                                rhs=qt_sb[0:DH, j, ts(t, QT)],
                                start=True, stop=True,
                            )
                            nc.tensor.matmul(
                                stp[:, i, 1, :],
                                lhsT=kt_sb[DH:P, j, ts(kc, P)],
                                rhs=qt_sb[DH:P, j, ts(t, QT)],
                                start=True, stop=True,
                            )
                        nc.scalar.activation(
                            out=es[:, 2 * g:2 * g + 2, :, :], in_=stp[:, :, :, :],
                            func=Exp, scale=1.0 / DH,
                        )
                    rcb = rc_pool.tile([P, QT], f32, tag="rcb")
                    avs = []
                    for jj in range(2):
                        h = 2 * j + jj
                        avp = av_psum.tile([P, QT], f32, tag="avp")
                        avs.append(avp)
                        for kc in range(NKT):
                            nc.tensor.matmul(
                                avp,
                                lhsT=vaug[:, kc, h, :],
                                rhs=es[:, kc, jj, :],
                                start=(kc == 0), stop=(kc == NKT - 1),
                            )
                    # normalize: O / denom (denom rows: 64 for even, 0 for odd).
                    # Partition-broadcast of the reciprocal row is done by
                    # bouncing through DRAM (SBUF APs may not have stride-0
                    # partition dims; DRAM APs may).
                    rcw = rc_pool.tile([P, QT], f32, tag="rcw")
                    nc.vector.reciprocal(rcw[DH:DH + 1, :], avs[0][DH:DH + 1, :])
                    nc.vector.reciprocal(rcw[0:1, :], avs[1][0:1, :])
                    den_d = dram_pool.tile([2, QT], f32, tag="dend")
                    nc.sync.dma_start(den_d[0, None, :], rcw[DH:DH + 1, :])
                    nc.sync.dma_start(den_d[1, None, :], rcw[0:1, :])
                    nc.sync.dma_start(
                        rcb[0:DH, :], den_d[0, None, :].to_broadcast((DH, QT)))
                    nc.sync.dma_start(
                        rcb[DH:P, :], den_d[1, None, :].to_broadcast((DH, QT)))
                    nc.vector.tensor_tensor(
                        out=ot[0:DH, j, :], in0=avs[0][0:DH, :],
                        in1=rcb[0:DH, :], op=MUL,
                    )
                    nc.vector.tensor_tensor(
                        out=ot[DH:P, j, :], in0=avs[1][DH:P, :],
                        in1=rcb[DH:P, :], op=MUL,
                    )

                # ---- partial output projection for this q-tile ----
                for tt in range(QT // P):
                    tok = t * (QT // P) + tt
                    for eh in range(DIM // 512):
                        ps = mm_psum.tile([P, 512], f32, tag="mmp")
                        for cc in range(CC):
                            nc.tensor.matmul(
                                ps, lhsT=ot[:, cc, ts(tt, P)],
                                rhs=wf_sb[:, cc, ts(eh, 512)],
                                start=(cc == 0), stop=(cc == CC - 1),
                            )
                        ysb = ysb_pool.tile([P, 512], f32, tag="ysb")
                        nc.vector.tensor_copy(out=ysb, in_=ps)
                        nc.sync.dma_start(y_r[tok, :, ts(eh, 512)], ysb)

    nc.compile()
    return nc


def _get_nc():
    if "nc" not in _CACHE:
        _CACHE["nc"] = _build()
    return _CACHE["nc"]


def kernel(**inputs):
    global LAST_RESULTS
    query = np.asarray(inputs["query"], np.float32)
    key = np.asarray(inputs["key"], np.float32)
    value = np.asarray(inputs["value"], np.float32)
    pad_mask = np.asarray(inputs["pad_mask"])
    training = int(np.asarray(inputs["training_status"]))
    Wq = np.asarray(inputs["Wq"], np.float32)
    Wk = np.asarray(inputs["Wk"], np.float32)
    Wv = np.asarray(inputs["Wv"], np.float32)
    Wf = np.asarray(inputs["Wf"], np.float32)
    bq = np.asarray(inputs["bq"], np.float32)
    bk = np.asarray(inputs["bk"], np.float32)
    bv = np.asarray(inputs["bv"], np.float32)
    bf = np.asarray(inputs["bf"], np.float32)

    nc = _get_nc()

    xT = {}
    for b in range(B):
        xT[("q", b)] = np.ascontiguousarray(query[b].T).astype(BF16)
        xT[("k", b)] = np.ascontiguousarray(key[b].T).astype(BF16)
        xT[("v", b)] = np.ascontiguousarray(value[b].T).astype(BF16)

    in_maps = []
    for c in range(NCORES):
        b, g = divmod(c, HPC)
        cs = slice(g * CSL, (g + 1) * CSL)
        if training:
            m01 = (pad_mask[b, 0, 0, :] != 0).astype(np.float32)
        else:
            m01 = np.ones(S, np.float32)
        in_maps.append({
            "xq": xT[("q", b)],
            "xk": xT[("k", b)],
            "xv": xT[("v", b)],
            "wq": np.ascontiguousarray(Wq[:, cs]).astype(BF16),
            "wk": np.ascontiguousarray(Wk[:, cs]).astype(BF16),
            "wv": np.ascontiguousarray(Wv[:, cs]).astype(BF16),
            "wf": np.ascontiguousarray(Wf[cs, :]).astype(BF16),
            "bq": np.ascontiguousarray(bq[cs]),
            "bk": np.ascontiguousarray(bk[cs]),
            "bv": np.ascontiguousarray(bv[cs]),
            "m01": m01,
        })

    from concourse.bass_utils import run_bass_kernel_spmd

    res = run_bass_kernel_spmd(nc, in_maps, core_ids=list(range(NCORES)))
    LAST_RESULTS = res

    out = np.zeros((B, S, DIM), np.float32)
    for c in range(NCORES):
        b = c // HPC
        out[b] += res.results[c]["y"]
    out += bf[None, None, :]
    return out


# revision 7
# speedup vs baseline: 1.2233x; 1.2233x over previous
"""Multi-head attention TRN2 Bass kernel, sharded over 8 NeuronCores.

Sharding: core c -> (batch b = c//4, head-group g = c%4).  Each core computes
4 heads' worth of Q/K/V projections + attention for one batch element, plus
the partial output projection for its 256-column slice of the head-concat
dimension.  Host sums the 4 partials per batch and adds bf.

Device-side structure (all matmuls bf16, fp32 PSUM):
  - Host pre-transposes x to [DIM, S] so contraction dims sit on partitions.
  - Scores are computed transposed (S^T[kt, qt]); the two heads of a pair use
    disjoint PE row halves (K=64 at base partitions 0/64) and run concurrently.
  - pad_mask is folded into V and into an extra mask-column of V, so the AV
    matmul produces the masked numerator AND the softmax denominator, and exp
    needs no mask bias (raw scores are tiny; exp cannot overflow).
  - Everything is software-pipelined per 128-wide key chunk so the ACT engine
    (exp) stays saturated while PE interleaves QK, AV, projections and the
    output projection of the previous query tile.
"""

import os
import numpy as np
import ml_dtypes

B, S, DIM, H, DH = 2, 2048, 1024, 16, 64
NCORES = 8
HPC = 4           # heads per core
CSL = HPC * DH    # 256: per-core slice of the head-concat dim
P = 128
KO = DIM // P     # 8 contraction chunks for projections
CC = CSL // P     # 2 col chunks (2 head-pairs)
NKT = S // P      # 16 key-token chunks
QT = 512          # query tile (free dim)
NQT = S // QT     # 4 query tiles
TOK = S // P      # 16 token chunks

BF16 = ml_dtypes.bfloat16

_CACHE = {}
LAST_RESULTS = None


def _build():
    import concourse.bass as bass
    import concourse.tile as tile
    from concourse import bacc, mybir
    from concourse.bass import ts

    f32 = mybir.dt.float32
    bf16 = mybir.dt.bfloat16

    nc = bacc.Bacc("TRN2", target_bir_lowering=False, debug=False)

    xq = nc.dram_tensor("xq", [DIM, S], bf16, kind="ExternalInput").ap()
    xk = nc.dram_tensor("xk", [DIM, S], bf16, kind="ExternalInput").ap()
    xv = nc.dram_tensor("xv", [DIM, S], bf16, kind="ExternalInput").ap()
    wq = nc.dram_tensor("wq", [DIM, CSL], bf16, kind="ExternalInput").ap()
    wk = nc.dram_tensor("wk", [DIM, CSL], bf16, kind="ExternalInput").ap()
    wv = nc.dram_tensor("wv", [DIM, CSL], bf16, kind="ExternalInput").ap()
    wf = nc.dram_tensor("wf", [CSL, DIM], bf16, kind="ExternalInput").ap()
    bq = nc.dram_tensor("bq", [CSL], f32, kind="ExternalInput").ap()
    bk = nc.dram_tensor("bk", [CSL], f32, kind="ExternalInput").ap()
    bv = nc.dram_tensor("bv", [CSL], f32, kind="ExternalInput").ap()
    m01 = nc.dram_tensor("m01", [S], f32, kind="ExternalInput").ap()
    y = nc.dram_tensor("y", [S, DIM], f32, kind="ExternalOutput").ap()

    Exp = mybir.ActivationFunctionType.Exp
    MUL = mybir.AluOpType.mult

    with tile.TileContext(nc) as tc:
        with (
            tc.tile_pool(name="const", bufs=1) as const,
            tc.tile_pool(name="xql", bufs=3) as xql_pool,
            tc.tile_pool(name="xvl", bufs=3) as xvl_pool,
            tc.tile_pool(name="qkv", bufs=1) as qkv,
            tc.tile_pool(name="es", bufs=2) as es_pool,
            tc.tile_pool(name="ot", bufs=2) as ot_pool,
            tc.tile_pool(name="ysb", bufs=3) as ysb_pool,
            tc.tile_pool(name="rc", bufs=2) as rc_pool,
            tc.tile_pool(name="dscr", bufs=2, space="DRAM") as dram_pool,
            tc.tile_pool(name="stp", bufs=2, space="PSUM") as st_psum,
            tc.tile_pool(name="avp", bufs=2, space="PSUM") as av_psum,
            tc.tile_pool(name="mmp", bufs=2, space="PSUM") as mm_psum,
        ):
            # ---- constants ----
            wq_sb = const.tile([P, KO, CSL], bf16)
            wk_sb = const.tile([P, KO, CSL], bf16)
            wv_sb = const.tile([P, KO, CSL], bf16)
            nc.sync.dma_start(wk_sb, wk.rearrange("(ko p) e -> p ko e", p=P))
            nc.sync.dma_start(wq_sb, wq.rearrange("(ko p) e -> p ko e", p=P))
            nc.sync.dma_start(wv_sb, wv.rearrange("(ko p) e -> p ko e", p=P))
            wf_sb = const.tile([P, CC, DIM], bf16)
            nc.sync.dma_start(wf_sb, wf.rearrange("(cc p) e -> p cc e", p=P))
            bq_sb = const.tile([P, CC], f32)
            bk_sb = const.tile([P, CC], f32)
            nc.sync.dma_start(bk_sb, bk.rearrange("(cc p) -> p cc", p=P))
            nc.sync.dma_start(bq_sb, bq.rearrange("(cc p) -> p cc", p=P))
            bv_sb = const.tile([P, CSL], f32)
            nc.sync.dma_start(bv_sb, bv[None, :].to_broadcast((P, CSL)))
            m01_sb = const.tile([P, NKT], f32)
            nc.sync.dma_start(m01_sb, m01.rearrange("(c p) -> p c", p=P))

            xq_r = xq.rearrange("(ko p) s -> p ko s", p=P)
            xk_r = xk.rearrange("(ko p) s -> p ko s", p=P)
            xv_r = xv.rearrange("(ko p) s -> p ko s", p=P)
            y_r = y.rearrange("(t p) e -> t p e", p=P)

            qt_sb = qkv.tile([P, CC, S], bf16)
            kt_sb = qkv.tile([P, CC, S], bf16)

            # V in AV-stationary form. Per head h, vaug[:, kc, h, :] is 128
            # wide: even h -> [V(64) | m01 | 0..], odd h -> [m01 | 0(63) | V(64)].
            # AV psum rows: even: O at 0..63, denom at 64;
            #               odd:  denom at 0, O at 64..127.
            vaug = qkv.tile([P, NKT, HPC, P], bf16)
            vaug_v = vaug.rearrange("p c (hp par) w -> p par c hp w", par=2)
            bv_v = bv_sb.rearrange("p (hp par d) -> p par hp d", par=2, d=DH)

            def emit_kq_proj(x_r, w_sb, b_sb, dst, t, ccs=(0, 1)):
                """One 512-token tile of the K^T / Q^T projection."""
                xt = xql_pool.tile([P, KO, QT], bf16, tag="xql")
                nc.sync.dma_start(xt, x_r[:, :, ts(t, QT)])
                for cc in ccs:
                    ps = mm_psum.tile([P, QT], f32, tag="mmp")
                    for ko in range(KO):
                        nc.tensor.matmul(
                            ps, lhsT=w_sb[:, ko, ts(cc, P)], rhs=xt[:, ko, :],
                            start=(ko == 0), stop=(ko == KO - 1),
                        )
                    nc.vector.tensor_add(
                        out=dst[:, cc, ts(t, QT)], in0=ps,
                        in1=b_sb[:, cc, None].to_broadcast((P, QT)),
                    )

            def emit_vproj_chunk(t):
                """One 128-token chunk of the V projection into vaug."""
                xt = xvl_pool.tile([P, KO, P], bf16, tag="xvl")
                nc.sync.dma_start(xt, xv_r[:, :, ts(t, P)])
                ps = mm_psum.tile([P, CSL], f32, tag="mmp")
                for ko in range(KO):
                    nc.tensor.matmul(
                        ps, lhsT=xt[:, ko, :], rhs=wv_sb[:, ko, :],
                        start=(ko == 0), stop=(ko == KO - 1),
                    )
                ps_v = ps.rearrange("p (hp par d) -> p par hp d", par=2, d=DH)
                for par, dlo in ((0, 0), (1, DH)):
                    dst = vaug_v[:, par, t, :, dlo:dlo + DH]
                    nc.vector.tensor_add(
                        out=dst, in0=ps_v[:, par, :, :], in1=bv_v[:, par, :, :],
                    )
                    nc.vector.tensor_tensor(
                        out=dst, in0=dst,
                        in1=m01_sb[:, t, None, None].to_broadcast((P, 2, DH)),
                        op=MUL,
                    )

            def emit_f_unit(t, tt, eh, ot):
                """One [128 tok, 512 e] block of the output projection."""
                tok = t * (QT // P) + tt
                ps = mm_psum.tile([P, 512], f32, tag="mmp")
                for cc in range(CC):
                    nc.tensor.matmul(
                        ps, lhsT=ot[:, cc, ts(tt, P)],
                        rhs=wf_sb[:, cc, ts(eh, 512)],
                        start=(cc == 0), stop=(cc == CC - 1),
                    )
                ysb = ysb_pool.tile([P, 512], f32, tag="ysb")
                nc.vector.tensor_copy(out=ysb, in_=ps)
                nc.sync.dma_start(y_r[tok, :, ts(eh, 512)], ysb)

            # ---- lead-in ----
            # vaug zero/mask columns (DVE work, overlaps the DMAs)
            nc.vector.memset(vaug, 0.0)
            nc.vector.tensor_copy(
                out=vaug_v[:, 0, :, :, DH],
                in_=m01_sb[:, :, None].to_broadcast((P, NKT, 2)),
            )
            nc.vector.tensor_copy(
                out=vaug_v[:, 1, :, :, 0],
                in_=m01_sb[:, :, None].to_broadcast((P, NKT, 2)),
            )
            for t in range(NQT):
                emit_kq_proj(xk_r, wk_sb, bk_sb, kt_sb, t)
            emit_kq_proj(xq_r, wq_sb, bq_sb, qt_sb, 0)

            AV_DELAY = 2

            def emit_pair(t, j, extra_units):
                """Score/exp/AV pipeline for head pair j of q-tile t.

                extra_units: callables emitting small PE work blocks, spread
                across the kc loop to keep PE busy during exp waits."""
                es = es_pool.tile([P, NKT, 2, QT], bf16, tag="es")
                avs = [av_psum.tile([P, QT], f32, tag="avp", name=f"avp{t}_{j}_{jj}") for jj in range(2)]

                def av_step(kc):
                    for jj in range(2):
                        nc.tensor.matmul(
                            avs[jj],
                            lhsT=vaug[:, kc, 2 * j + jj, :],
                            rhs=es[:, kc, jj, :],
                            start=(kc == 0), stop=(kc == NKT - 1),
                        )

                nu = len(extra_units)
                ei = 0
                for kc in range(NKT):
                    stp = st_psum.tile([P, 2, QT], f32, tag="stp")
                    nc.tensor.matmul(
                        stp[:, 0, :],
                        lhsT=kt_sb[0:DH, j, ts(kc, P)],
                        rhs=qt_sb[0:DH, j, ts(t, QT)],
                        start=True, stop=True,
                    )
                    nc.tensor.matmul(
                        stp[:, 1, :],
                        lhsT=kt_sb[DH:P, j, ts(kc, P)],
                        rhs=qt_sb[DH:P, j, ts(t, QT)],
                        start=True, stop=True,
                    )
                    nc.scalar.activation(
                        out=es[:, kc, :, :], in_=stp[:, :, :],
                        func=Exp, scale=1.0 / DH,
                    )
                    # spread extra units evenly over the kc loop
                    target = (kc + 1) * nu // NKT
                    while ei < target:
                        extra_units[ei]()
                        ei += 1
                    if kc >= AV_DELAY:
                        av_step(kc - AV_DELAY)
                for kc in range(NKT - AV_DELAY, NKT):
                    av_step(kc)

                # normalize: ot[0:64] = O_even * 1/denom_even (denom row 64),
                #            ot[64:128] = O_odd * 1/denom_odd (denom row 0).
                # Partition-replication of the reciprocal row goes via DRAM
                # (only DRAM APs may have stride-0 partition dims).
                rcw = rc_pool.tile([P, QT], f32, tag="rcw")
                rcb = rc_pool.tile([P, QT], f32, tag="rcb")
                nc.vector.reciprocal(rcw[DH:DH + 1, :], avs[0][DH:DH + 1, :])
                nc.vector.reciprocal(rcw[0:1, :], avs[1][0:1, :])
                den_d = dram_pool.tile([2, QT], f32, tag="dend")
                nc.sync.dma_start(den_d[0, None, :], rcw[DH:DH + 1, :])
                nc.sync.dma_start(den_d[1, None, :], rcw[0:1, :])
                nc.sync.dma_start(
                    rcb[0:DH, :], den_d[0, None, :].to_broadcast((DH, QT)))
                nc.sync.dma_start(
                    rcb[DH:P, :], den_d[1, None, :].to_broadcast((DH, QT)))
                nc.vector.tensor_tensor(
                    out=ots[t][0:DH, j, :], in0=avs[0][0:DH, :],
                    in1=rcb[0:DH, :], op=MUL,
                )
                nc.vector.tensor_tensor(
                    out=ots[t][DH:P, j, :], in0=avs[1][DH:P, :],
                    in1=rcb[DH:P, :], op=MUL,
                )

            ots = {}
            for t in range(NQT):
                ots[t] = ot_pool.tile([P, CC, QT], bf16, tag="ot", name=f"ot{t}")
                if t == 0:
                    extra0 = [
                        (lambda tt=tt: emit_vproj_chunk(tt)) for tt in range(TOK)
                    ]
                else:
                    extra0 = [
                        (lambda tt=tt, eh=eh, tp=t - 1: emit_f_unit(tp, tt, eh, ots[tp]))
                        for tt in range(QT // P) for eh in range(2)
                    ]
                emit_pair(t, 0, extra0)
                if t < NQT - 1:
                    extra1 = [
                        (lambda cc=cc, tn=t + 1: emit_kq_proj(
                            xq_r, wq_sb, bq_sb, qt_sb, tn, ccs=(cc,)))
                        for cc in range(CC)
                    ]
                else:
                    extra1 = []
                emit_pair(t, 1, extra1)
            # tail: output projection of the last q-tile
            for tt in range(QT // P):
                for eh in range(2):
                    emit_f_unit(NQT - 1, tt, eh, ots[NQT - 1])

    nc.compile()
    return nc


def _get_nc():
    if "nc" not in _CACHE:
        _CACHE["nc"] = _build()
    return _CACHE["nc"]


def kernel(**inputs):
    global LAST_RESULTS
    query = np.asarray(inputs["query"], np.float32)
    key = np.asarray(inputs["key"], np.float32)
    value = np.asarray(inputs["value"], np.float32)
    pad_mask = np.asarray(inputs["pad_mask"])
    training = int(np.asarray(inputs["training_status"]))
    Wq = np.asarray(inputs["Wq"], np.float32)
    Wk = np.asarray(inputs["Wk"], np.float32)
    Wv = np.asarray(inputs["Wv"], np.float32)
    Wf = np.asarray(inputs["Wf"], np.float32)
    bq = np.asarray(inputs["bq"], np.float32)
    bk = np.asarray(inputs["bk"], np.float32)
    bv = np.asarray(inputs["bv"], np.float32)
    bf = np.asarray(inputs["bf"], np.float32)

    nc = _get_nc()

    xT = {}
    for b in range(B):
        xT[("q", b)] = np.ascontiguousarray(query[b].T).astype(BF16)
        xT[("k", b)] = np.ascontiguousarray(key[b].T).astype(BF16)
        xT[("v", b)] = np.ascontiguousarray(value[b].T).astype(BF16)

    in_maps = []
    for c in range(NCORES):
        b, g = divmod(c, HPC)
        cs = slice(g * CSL, (g + 1) * CSL)
        if training:
            m01 = (pad_mask[b, 0, 0, :] != 0).astype(np.float32)
        else:
            m01 = np.ones(S, np.float32)
        in_maps.append({
            "xq": xT[("q", b)],
            "xk": xT[("k", b)],
            "xv": xT[("v", b)],
            "wq": np.ascontiguousarray(Wq[:, cs]).astype(BF16),
            "wk": np.ascontiguousarray(Wk[:, cs]).astype(BF16),
            "wv": np.ascontiguousarray(Wv[:, cs]).astype(BF16),
            "wf": np.ascontiguousarray(Wf[cs, :]).astype(BF16),
            "bq": np.ascontiguousarray(bq[cs]),
            "bk": np.ascontiguousarray(bk[cs]),
            "bv": np.ascontiguousarray(bv[cs]),
            "m01": m01,
        })

    from concourse.bass_utils import run_bass_kernel_spmd

    res = run_bass_kernel_spmd(nc, in_maps, core_ids=list(range(NCORES)))
    LAST_RESULTS = res

    out = np.zeros((B, S, DIM), np.float32)
    for c in range(NCORES):
        b = c // HPC
        out[b] += res.results[c]["y"]
    out += bf[None, None, :]
    return out


# revision 10
# speedup vs baseline: 1.4173x; 1.1585x over previous
"""Multi-head attention TRN2 Bass kernel, sharded over 8 NeuronCores.

Sharding: core c -> (batch b = c//4, head-group g = c%4).  Each core computes
4 heads' worth of Q/K/V projections + attention for one batch element, plus
the partial output projection for its 256-column slice of the head-concat
dimension.  Host sums the 4 partials per batch and adds bf.

Key tricks:
  - All matmuls bf16 with fp32 PSUM accumulation.
  - Host pre-transposes x to [DIM, S] so contraction dims sit on partitions.
  - Attention is permutation-invariant over keys: the host sorts keys so
    unmasked tokens come first, and the kernel only processes the first
    NKT_A 128-token key chunks (fully-masked chunks contribute exactly 0).
  - Scores are computed transposed (S^T[kt, qt]); the two heads of a pair use
    disjoint PE row halves (K=64 at base partitions 0/64) and run concurrently.
  - pad_mask is folded into V and into an extra mask-column of V, so the AV
    matmul produces the masked numerator AND the softmax denominator, and exp
    needs no mask bias (raw scores are tiny; exp cannot overflow).
  - Fine-grained software pipelining: each head-pair's QK/exp loop also
    carries the previous pair's AV accumulation plus small projection /
    output-projection work units, keeping PE dense.
"""

import os
import numpy as np
import ml_dtypes

B, S, DIM, H, DH = 2, 2048, 1024, 16, 64
NCORES = 8
HPC = 4           # heads per core
CSL = HPC * DH    # 256: per-core slice of the head-concat dim
P = 128
KO = DIM // P     # 8 contraction chunks for projections
CC = CSL // P     # 2 col chunks (2 head-pairs)
NKT = S // P      # 16 key-token chunks (full)
QT = 512          # query tile (free dim)
NQT = S // QT     # 4 query tiles

BF16 = ml_dtypes.bfloat16

_CACHE = {}
LAST_RESULTS = None


def _build(nkt_a):
    import concourse.bass as bass
    import concourse.tile as tile
    from concourse import bacc, mybir
    from concourse.bass import ts

    f32 = mybir.dt.float32
    bf16 = mybir.dt.bfloat16

    KTILES = (nkt_a + 3) // 4          # 512-token K-projection tiles
    KTOK = KTILES * QT                 # padded key-token extent

    nc = bacc.Bacc("TRN2", target_bir_lowering=False, debug=False)

    xq = nc.dram_tensor("xq", [DIM, S], bf16, kind="ExternalInput").ap()
    xk = nc.dram_tensor("xk", [DIM, KTOK], bf16, kind="ExternalInput").ap()
    xv = nc.dram_tensor("xv", [DIM, KTOK], bf16, kind="ExternalInput").ap()
    wq = nc.dram_tensor("wq", [DIM, CSL], bf16, kind="ExternalInput").ap()
    wk = nc.dram_tensor("wk", [DIM, CSL], bf16, kind="ExternalInput").ap()
    wv = nc.dram_tensor("wv", [DIM, CSL], bf16, kind="ExternalInput").ap()
    wf = nc.dram_tensor("wf", [CSL, DIM], bf16, kind="ExternalInput").ap()
    bq = nc.dram_tensor("bq", [CSL], f32, kind="ExternalInput").ap()
    bk = nc.dram_tensor("bk", [CSL], f32, kind="ExternalInput").ap()
    bv = nc.dram_tensor("bv", [CSL], f32, kind="ExternalInput").ap()
    m01 = nc.dram_tensor("m01", [nkt_a * P], f32, kind="ExternalInput").ap()
    y = nc.dram_tensor("y", [S, DIM], f32, kind="ExternalOutput").ap()

    Exp = mybir.ActivationFunctionType.Exp
    MUL = mybir.AluOpType.mult

    with tile.TileContext(nc) as tc:
        with (
            tc.tile_pool(name="const", bufs=1) as const,
            tc.tile_pool(name="xql", bufs=3) as xql_pool,
            tc.tile_pool(name="xvl", bufs=3) as xvl_pool,
            tc.tile_pool(name="qkv", bufs=1) as qkv,
            tc.tile_pool(name="es", bufs=2) as es_pool,
            tc.tile_pool(name="ot", bufs=2) as ot_pool,
            tc.tile_pool(name="ysb", bufs=3) as ysb_pool,
            tc.tile_pool(name="rc", bufs=2) as rc_pool,
            tc.tile_pool(name="dscr", bufs=2, space="DRAM") as dram_pool,
            tc.tile_pool(name="stp", bufs=2, space="PSUM") as st_psum,
            tc.tile_pool(name="avp", bufs=2, space="PSUM") as av_psum,
            tc.tile_pool(name="mmp", bufs=2, space="PSUM") as mm_psum,
        ):
            # ---- constants ----
            wk_sb = const.tile([P, KO, CSL], bf16)
            wq_sb = const.tile([P, KO, CSL], bf16)
            wv_sb = const.tile([P, KO, CSL], bf16)
            nc.sync.dma_start(wk_sb, wk.rearrange("(ko p) e -> p ko e", p=P))
            nc.sync.dma_start(wq_sb, wq.rearrange("(ko p) e -> p ko e", p=P))
            nc.sync.dma_start(wv_sb, wv.rearrange("(ko p) e -> p ko e", p=P))
            wf_sb = const.tile([P, CC, DIM], bf16)
            nc.sync.dma_start(wf_sb, wf.rearrange("(cc p) e -> p cc e", p=P))
            bk_sb = const.tile([P, CC], f32)
            bq_sb = const.tile([P, CC], f32)
            nc.sync.dma_start(bk_sb, bk.rearrange("(cc p) -> p cc", p=P))
            nc.sync.dma_start(bq_sb, bq.rearrange("(cc p) -> p cc", p=P))
            bv_sb = const.tile([P, CSL], f32)
            nc.sync.dma_start(bv_sb, bv[None, :].to_broadcast((P, CSL)))
            m01_sb = const.tile([P, nkt_a], f32)
            nc.sync.dma_start(m01_sb, m01.rearrange("(c p) -> p c", p=P))

            xq_r = xq.rearrange("(ko p) s -> p ko s", p=P)
            xk_r = xk.rearrange("(ko p) s -> p ko s", p=P)
            xv_r = xv.rearrange("(ko p) s -> p ko s", p=P)
            y_r = y.rearrange("(t p) e -> t p e", p=P)

            qt_sb = qkv.tile([P, CC, S], bf16)
            kt_sb = qkv.tile([P, CC, KTOK], bf16)

            # V in AV-stationary form. Per head h, vaug[:, kc, h, :] is 128
            # wide: even h -> [V(64) | m01 | 0..], odd h -> [m01 | 0(63) | V(64)].
            # AV psum rows: even: O at 0..63, denom at 64;
            #               odd:  denom at 0, O at 64..127.
            vaug = qkv.tile([P, nkt_a, HPC, P], bf16)
            vaug_v = vaug.rearrange("p c (hp par) w -> p par c hp w", par=2)
            bv_v = bv_sb.rearrange("p (hp par d) -> p par hp d", par=2, d=DH)

            def emit_kq_proj(x_r, w_sb, b_sb, dst, t, cc):
                """One (512-token, 128-col) block of the K^T / Q^T projection."""
                xt = xql_pool.tile([P, KO, QT], bf16, tag="xql",
                                   name=f"x{dst.tensor.name[:2]}_{t}_{cc}")
                nc.sync.dma_start(xt, x_r[:, :, ts(t, QT)])
                ps = mm_psum.tile([P, QT], f32, tag="mmp", name=f"pp{t}_{cc}")
                for ko in range(KO):
                    nc.tensor.matmul(
                        ps, lhsT=w_sb[:, ko, ts(cc, P)], rhs=xt[:, ko, :],
                        start=(ko == 0), stop=(ko == KO - 1),
                    )
                nc.vector.tensor_add(
                    out=dst[:, cc, ts(t, QT)], in0=ps,
                    in1=b_sb[:, cc, None].to_broadcast((P, QT)),
                )

            def emit_vproj_chunk(t):
                """One 128-token chunk of the V projection into vaug."""
                xt = xvl_pool.tile([P, KO, P], bf16, tag="xvl", name=f"xv_{t}")
                nc.sync.dma_start(xt, xv_r[:, :, ts(t, P)])
                ps = mm_psum.tile([P, CSL], f32, tag="mmp", name=f"vp{t}")
                for ko in range(KO):
                    nc.tensor.matmul(
                        ps, lhsT=xt[:, ko, :], rhs=wv_sb[:, ko, :],
                        start=(ko == 0), stop=(ko == KO - 1),
                    )
                ps_v = ps.rearrange("p (hp par d) -> p par hp d", par=2, d=DH)
                for par, dlo in ((0, 0), (1, DH)):
                    dst = vaug_v[:, par, t, :, dlo:dlo + DH]
                    nc.vector.tensor_add(
                        out=dst, in0=ps_v[:, par, :, :], in1=bv_v[:, par, :, :],
                    )
                    nc.vector.tensor_tensor(
                        out=dst, in0=dst,
                        in1=m01_sb[:, t, None, None].to_broadcast((P, 2, DH)),
                        op=MUL,
                    )

            def emit_f_unit(t, tt, eh):
                """One [128 tok, 512 e] block of the output projection."""
                tok = t * (QT // P) + tt
                ps = mm_psum.tile([P, 512], f32, tag="mmp", name=f"fp{tok}_{eh}")
                for cc in range(CC):
                    nc.tensor.matmul(
                        ps, lhsT=ots[t][:, cc, ts(tt, P)],
                        rhs=wf_sb[:, cc, ts(eh, 512)],
                        start=(cc == 0), stop=(cc == CC - 1),
                    )
                ysb = ysb_pool.tile([P, 512], f32, tag="ysb", name=f"ys{tok}_{eh}")
                nc.vector.tensor_copy(out=ysb, in_=ps)
                nc.sync.dma_start(y_r[tok, :, ts(eh, 512)], ysb)

            class PairState:
                """QK/exp products of one head pair, awaiting AV drain."""

                def __init__(self, t, j):
                    self.t, self.j = t, j
                    self.es = es_pool.tile([P, nkt_a, 2, QT], bf16, tag="es",
                                           name=f"es{t}_{j}")
                    self.avs = [
                        av_psum.tile([P, QT], f32, tag="avp",
                                     name=f"avp{t}_{j}_{jj}")
                        for jj in range(2)
                    ]
                    self.av_kc = 0

                def av_step(self):
                    kc = self.av_kc
                    for jj in range(2):
                        nc.tensor.matmul(
                            self.avs[jj],
                            lhsT=vaug[:, kc, 2 * self.j + jj, :],
                            rhs=self.es[:, kc, jj, :],
                            start=(kc == 0), stop=(kc == nkt_a - 1),
                        )
                    self.av_kc += 1

                def av_drain(self, upto):
                    while self.av_kc < upto:
                        self.av_step()

                def normalize(self):
                    t, j = self.t, self.j
                    rcw = rc_pool.tile([P, QT], f32, tag="rcw", name=f"rw{t}{j}")
                    rcb = rc_pool.tile([P, QT], f32, tag="rcb", name=f"rb{t}{j}")
                    nc.vector.reciprocal(
                        rcw[DH:DH + 1, :], self.avs[0][DH:DH + 1, :])
                    nc.vector.reciprocal(rcw[0:1, :], self.avs[1][0:1, :])
                    den_d = dram_pool.tile([2, QT], f32, tag="dend",
                                           name=f"dd{t}{j}")
                    nc.sync.dma_start(den_d[0, None, :], rcw[DH:DH + 1, :])
                    nc.sync.dma_start(den_d[1, None, :], rcw[0:1, :])
                    nc.sync.dma_start(
                        rcb[0:DH, :], den_d[0, None, :].to_broadcast((DH, QT)))
                    nc.sync.dma_start(
                        rcb[DH:P, :], den_d[1, None, :].to_broadcast((DH, QT)))
                    nc.vector.tensor_tensor(
                        out=ots[t][0:DH, j, :], in0=self.avs[0][0:DH, :],
                        in1=rcb[0:DH, :], op=MUL,
                    )
                    nc.vector.tensor_tensor(
                        out=ots[t][DH:P, j, :], in0=self.avs[1][DH:P, :],
                        in1=rcb[DH:P, :], op=MUL,
                    )

            def emit_pair(t, j, units, drain=None, self_av=False):
                """QK+exp loop for pair (t, j), interleaving `units` and the
                AV drain of a previous pair (and optionally its own)."""
                st = PairState(t, j)
                nu = len(units)
                ei = 0
                for kc in range(nkt_a):
                    stp = st_psum.tile([P, 2, QT], f32, tag="stp",
                                       name=f"st{t}_{j}_{kc}")
                    nc.tensor.matmul(
                        stp[:, 0, :],
                        lhsT=kt_sb[0:DH, j, ts(kc, P)],
                        rhs=qt_sb[0:DH, j, ts(t, QT)],
                        start=True, stop=True,
                    )
                    nc.tensor.matmul(
                        stp[:, 1, :],
                        lhsT=kt_sb[DH:P, j, ts(kc, P)],
                        rhs=qt_sb[DH:P, j, ts(t, QT)],
                        start=True, stop=True,
                    )
                    nc.scalar.activation(
                        out=st.es[:, kc, :, :], in_=stp[:, :, :],
                        func=Exp, scale=1.0 / DH,
                    )
                    target = (kc + 1) * nu // nkt_a
                    while ei < target:
                        units[ei]()
                        ei += 1
                    if drain is not None:
                        drain.av_drain(kc + 1)
                if drain is not None:
                    drain.av_drain(nkt_a)
                    drain.normalize()
                if self_av:
                    st.av_drain(nkt_a)
                    st.normalize()
                return st

            # ---- lead-in: just enough K/Q projection for the first pair ----
            emit_kq_proj(xk_r, wk_sb, bk_sb, kt_sb, 0, 0)
            emit_kq_proj(xq_r, wq_sb, bq_sb, qt_sb, 0, 0)
            nc.vector.memset(vaug, 0.0)
            nc.vector.tensor_copy(
                out=vaug_v[:, 0, :, :, DH],
                in_=m01_sb[:, :, None].to_broadcast((P, nkt_a, 2)),
            )
            nc.vector.tensor_copy(
                out=vaug_v[:, 1, :, :, 0],
                in_=m01_sb[:, :, None].to_broadcast((P, nkt_a, 2)),
            )

            ots = {
                t: ot_pool.tile([P, CC, QT], bf16, tag="ot", name=f"ot{t}")
                for t in range(NQT)
            }

            # remaining projection blocks as interleavable units
            k_units = [
                (lambda tt=tt, cc=cc: emit_kq_proj(xk_r, wk_sb, bk_sb, kt_sb, tt, cc))
                for cc in range(CC) for tt in range(KTILES) if not (tt == 0 and cc == 0)
            ]
            q0c1 = [lambda: emit_kq_proj(xq_r, wq_sb, bq_sb, qt_sb, 0, 1)]
            v_units = [
                (lambda tt=tt: emit_vproj_chunk(tt)) for tt in range(nkt_a)
            ]

            def qproj_units(t):
                return [
                    (lambda cc=cc, tn=t: emit_kq_proj(
                        xq_r, wq_sb, bq_sb, qt_sb, tn, cc))
                    for cc in range(CC)
                ]

            def f_units(t):
                return [
                    (lambda tt=tt, eh=eh, tp=t: emit_f_unit(tp, tt, eh))
                    for tt in range(QT // P) for eh in range(2)
                ]

            # Unit placement: ot(t-1) is complete only at the END of pair
            # (t, 0) (which drains pair (t-1, 1)), so f(t-1) units go in pair
            # (t, 1).  Qproj(t+1) must precede pair (t+1, 0): put it in (t, 0).
            prev = None
            for t in range(NQT):
                if t == 0:
                    u0 = k_units + q0c1 + qproj_units(1)
                    u1 = v_units
                else:
                    u0 = qproj_units(t + 1) if t < NQT - 1 else []
                    u1 = f_units(t - 1)
                p0 = emit_pair(t, 0, u0, drain=prev)
                p1 = emit_pair(t, 1, u1, drain=p0,
                               self_av=(t == NQT - 1))
                prev = p1
            # tail: last pair's AV ran self_av; finish f of the last two tiles
            for tt in range(QT // P):
                for eh in range(2):
                    emit_f_unit(NQT - 1, tt, eh)

    nc.compile()
    return nc


def _get_nc(nkt_a):
    if nkt_a not in _CACHE:
        _CACHE[nkt_a] = _build(nkt_a)
    return _CACHE[nkt_a]


def kernel(**inputs):
    global LAST_RESULTS
    query = np.asarray(inputs["query"], np.float32)
    key = np.asarray(inputs["key"], np.float32)
    value = np.asarray(inputs["value"], np.float32)
    pad_mask = np.asarray(inputs["pad_mask"])
    training = int(np.asarray(inputs["training_status"]))
    Wq = np.asarray(inputs["Wq"], np.float32)
    Wk = np.asarray(inputs["Wk"], np.float32)
    Wv = np.asarray(inputs["Wv"], np.float32)
    Wf = np.asarray(inputs["Wf"], np.float32)
    bq = np.asarray(inputs["bq"], np.float32)
    bk = np.asarray(inputs["bk"], np.float32)
    bv = np.asarray(inputs["bv"], np.float32)
    bf = np.asarray(inputs["bf"], np.float32)

    # Per-batch key permutation: unmasked keys first.  Attention is
    # permutation-invariant over keys, and fully-masked key chunks contribute
    # exactly zero (mask is folded into V and the denominator column), so the
    # kernel only needs ceil(max_unmasked / 128) key chunks.
    m01_full = {}
    perms = {}
    n_act = 1
    for b in range(B):
        if training:
            m = (pad_mask[b, 0, 0, :] != 0).astype(np.float32)
        else:
            m = np.ones(S, np.float32)
        perm = np.argsort(-m, kind="stable")
        m01_full[b] = m[perm]
        perms[b] = perm
        n_act = max(n_act, int(np.ceil(m.sum() / P)))
    nkt_a = min(NKT, max(2, n_act))
    ktok = ((nkt_a + 3) // 4) * QT

    nc = _get_nc(nkt_a)

    def prep_kv(x, b):
        xp = x[b][perms[b]]  # [S, DIM] permuted
        out = np.zeros((ktok, DIM), np.float32)
        out[: min(ktok, S)] = xp[:ktok]
        return np.ascontiguousarray(out.T).astype(BF16)

    xT = {}
    for b in range(B):
        xT[("q", b)] = np.ascontiguousarray(query[b].T).astype(BF16)
        xT[("k", b)] = prep_kv(key, b)
        xT[("v", b)] = prep_kv(value, b)
        m = np.zeros(nkt_a * P, np.float32)
        n = min(nkt_a * P, S)
        m[:n] = m01_full[b][:n]
        m01_full[b] = m

    in_maps = []
    for c in range(NCORES):
        b, g = divmod(c, HPC)
        cs = slice(g * CSL, (g + 1) * CSL)
        in_maps.append({
            "xq": xT[("q", b)],
            "xk": xT[("k", b)],
            "xv": xT[("v", b)],
            "wq": np.ascontiguousarray(Wq[:, cs]).astype(BF16),
            "wk": np.ascontiguousarray(Wk[:, cs]).astype(BF16),
            "wv": np.ascontiguousarray(Wv[:, cs]).astype(BF16),
            "wf": np.ascontiguousarray(Wf[cs, :]).astype(BF16),
            "bq": np.ascontiguousarray(bq[cs]),
            "bk": np.ascontiguousarray(bk[cs]),
            "bv": np.ascontiguousarray(bv[cs]),
            "m01": m01_full[b],
        })

    from concourse.bass_utils import run_bass_kernel_spmd

    res = run_bass_kernel_spmd(nc, in_maps, core_ids=list(range(NCORES)))
    LAST_RESULTS = res

    out = np.zeros((B, S, DIM), np.float32)
    for c in range(NCORES):
        b = c // HPC
        out[b] += res.results[c]["y"]
    out += bf[None, None, :]
    return out


# revision 14
# speedup vs baseline: 1.8460x; 1.3025x over previous
"""Multi-head attention TRN2 Bass kernel, sharded over 8 NeuronCores.

Sharding: core c -> (batch b = c//4, head-group g = c%4).  Each core computes
4 heads' worth of Q/K/V projections + attention for one batch element, plus
the partial output projection for its 256-column slice of the head-concat
dimension.  Host sums the 4 partials per batch and adds bf.

Key tricks:
  - All matmuls bf16 with fp32 PSUM accumulation.
  - Host pre-transposes x to [DIM, S] so contraction dims sit on partitions.
  - Attention is permutation-invariant over keys: the host sorts keys so
    unmasked tokens come first, and the kernel only processes the first
    NKT_A 128-token key chunks (fully-masked chunks contribute exactly 0).
  - Scores are computed transposed (S^T[kt, qt]); the two heads of a pair use
    disjoint PE row halves (K=64 at base partitions 0/64) and run concurrently.
  - pad_mask is folded into V and into an extra mask-column of V, so the AV
    matmul produces the masked numerator AND the softmax denominator, and exp
    needs no mask bias (raw scores are tiny; exp cannot overflow).
  - Fine-grained software pipelining: each head-pair's QK/exp loop also
    carries the previous pair's AV accumulation plus small projection /
    output-projection work units, keeping PE dense.
"""

import os
import numpy as np
import ml_dtypes

B, S, DIM, H, DH = 2, 2048, 1024, 16, 64
NCORES = 8
HPC = 4           # heads per core
CSL = HPC * DH    # 256: per-core slice of the head-concat dim
P = 128
KO = DIM // P     # 8 contraction chunks for projections
CC = CSL // P     # 2 col chunks (2 head-pairs)
NKT = S // P      # 16 key-token chunks (full)
QT = 512          # query tile (free dim)
NQT = S // QT     # 4 query tiles

BF16 = ml_dtypes.bfloat16

_CACHE = {}
LAST_RESULTS = None


def _build(nkt_a):
    import concourse.bass as bass
    import concourse.tile as tile
    from concourse import bacc, mybir
    from concourse.bass import ts

    f32 = mybir.dt.float32
    bf16 = mybir.dt.bfloat16

    KTILES = (nkt_a + 3) // 4          # 512-token K-projection tiles
    KTOK = KTILES * QT                 # padded key-token extent

    nc = bacc.Bacc("TRN2", target_bir_lowering=False, debug=False)

    xq = nc.dram_tensor("xq", [DIM, S], bf16, kind="ExternalInput").ap()
    xk = nc.dram_tensor("xk", [DIM, KTOK], bf16, kind="ExternalInput").ap()
    xv = nc.dram_tensor("xv", [DIM, KTOK], bf16, kind="ExternalInput").ap()
    wq = nc.dram_tensor("wq", [DIM, CSL], bf16, kind="ExternalInput").ap()
    wk = nc.dram_tensor("wk", [DIM, CSL], bf16, kind="ExternalInput").ap()
    wv = nc.dram_tensor("wv", [DIM, CSL], bf16, kind="ExternalInput").ap()
    wf = nc.dram_tensor("wf", [CSL, DIM], bf16, kind="ExternalInput").ap()
    bq = nc.dram_tensor("bq", [CSL], f32, kind="ExternalInput").ap()
    bk = nc.dram_tensor("bk", [CSL], f32, kind="ExternalInput").ap()
    bv = nc.dram_tensor("bv", [CSL], f32, kind="ExternalInput").ap()
    m01 = nc.dram_tensor("m01", [nkt_a * P], f32, kind="ExternalInput").ap()
    y = nc.dram_tensor("y", [S, DIM], f32, kind="ExternalOutput").ap()

    Exp = mybir.ActivationFunctionType.Exp
    MUL = mybir.AluOpType.mult

    with tile.TileContext(nc) as tc:
        with (
            tc.tile_pool(name="const", bufs=1) as const,
            tc.tile_pool(name="xql", bufs=3) as xql_pool,
            tc.tile_pool(name="xvl", bufs=3) as xvl_pool,
            tc.tile_pool(name="qkv", bufs=1) as qkv,
            tc.tile_pool(name="es", bufs=2) as es_pool,
            tc.tile_pool(name="ot", bufs=2) as ot_pool,
            tc.tile_pool(name="ysb", bufs=3) as ysb_pool,
            tc.tile_pool(name="rc", bufs=2) as rc_pool,
            tc.tile_pool(name="dscr", bufs=2, space="DRAM") as dram_pool,
            tc.tile_pool(name="stp", bufs=2, space="PSUM") as st_psum,
            tc.tile_pool(name="avp", bufs=2, space="PSUM") as av_psum,
            tc.tile_pool(name="mmp", bufs=2, space="PSUM") as mm_psum,
        ):
            # ---- constants ----
            wk_sb = const.tile([P, KO, CSL], bf16)
            wq_sb = const.tile([P, KO, CSL], bf16)
            wv_sb = const.tile([P, KO, CSL], bf16)
            nc.sync.dma_start(wk_sb, wk.rearrange("(ko p) e -> p ko e", p=P))
            nc.sync.dma_start(wq_sb, wq.rearrange("(ko p) e -> p ko e", p=P))
            nc.sync.dma_start(wv_sb, wv.rearrange("(ko p) e -> p ko e", p=P))
            wf_sb = const.tile([P, CC, DIM], bf16)
            nc.sync.dma_start(wf_sb, wf.rearrange("(cc p) e -> p cc e", p=P))
            bk_sb = const.tile([P, CC], f32)
            bq_sb = const.tile([P, CC], f32)
            nc.sync.dma_start(bk_sb, bk.rearrange("(cc p) -> p cc", p=P))
            nc.sync.dma_start(bq_sb, bq.rearrange("(cc p) -> p cc", p=P))
            bv_sb = const.tile([P, CSL], f32)
            nc.sync.dma_start(bv_sb, bv[None, :].to_broadcast((P, CSL)))
            m01_sb = const.tile([P, nkt_a], f32)
            nc.sync.dma_start(m01_sb, m01.rearrange("(c p) -> p c", p=P))

            xq_r = xq.rearrange("(ko p) s -> p ko s", p=P)
            xk_r = xk.rearrange("(ko p) s -> p ko s", p=P)
            xv_r = xv.rearrange("(ko p) s -> p ko s", p=P)
            y_r = y.rearrange("(t p) e -> t p e", p=P)

            qt_sb = qkv.tile([P, CC, S], bf16)
            kt_sb = qkv.tile([P, CC, KTOK], bf16)

            # V in AV-stationary form. Per head h, vaug[:, kc, h, :] is 128
            # wide: even h -> [V(64) | m01 | 0..], odd h -> [m01 | 0(63) | V(64)].
            # AV psum rows: even: O at 0..63, denom at 64;
            #               odd:  denom at 0, O at 64..127.
            vaug = qkv.tile([P, nkt_a, HPC, P], bf16)
            vaug_v = vaug.rearrange("p c (hp par) w -> p par c hp w", par=2)
            bv_v = bv_sb.rearrange("p (hp par d) -> p par hp d", par=2, d=DH)

            def emit_kq_proj(x_r, w_sb, b_sb, dst, t, cc):
                """One (512-token, 128-col) block of the K^T / Q^T projection."""
                xt = xql_pool.tile([P, KO, QT], bf16, tag="xql",
                                   name=f"x{dst.tensor.name[:2]}_{t}_{cc}")
                nc.sync.dma_start(xt, x_r[:, :, ts(t, QT)])
                ps = mm_psum.tile([P, QT], f32, tag="mmp", name=f"pp{t}_{cc}")
                for ko in range(KO):
                    nc.tensor.matmul(
                        ps, lhsT=w_sb[:, ko, ts(cc, P)], rhs=xt[:, ko, :],
                        start=(ko == 0), stop=(ko == KO - 1),
                    )
                nc.vector.tensor_add(
                    out=dst[:, cc, ts(t, QT)], in0=ps,
                    in1=b_sb[:, cc, None].to_broadcast((P, QT)),
                )

            def emit_vproj_chunk(t):
                """One 128-token chunk of the V projection into vaug."""
                xt = xvl_pool.tile([P, KO, P], bf16, tag="xvl", name=f"xv_{t}")
                nc.sync.dma_start(xt, xv_r[:, :, ts(t, P)])
                ps = mm_psum.tile([P, CSL], f32, tag="mmp", name=f"vp{t}")
                for ko in range(KO):
                    nc.tensor.matmul(
                        ps, lhsT=xt[:, ko, :], rhs=wv_sb[:, ko, :],
                        start=(ko == 0), stop=(ko == KO - 1),
                    )
                ps_v = ps.rearrange("p (hp par d) -> p par hp d", par=2, d=DH)
                for par, dlo in ((0, 0), (1, DH)):
                    dst = vaug_v[:, par, t, :, dlo:dlo + DH]
                    nc.vector.tensor_add(
                        out=dst, in0=ps_v[:, par, :, :], in1=bv_v[:, par, :, :],
                    )
                    nc.vector.tensor_tensor(
                        out=dst, in0=dst,
                        in1=m01_sb[:, t, None, None].to_broadcast((P, 2, DH)),
                        op=MUL,
                    )

            def emit_f_unit(t, tt, eh):
                """One [128 tok, 512 e] block of the output projection."""
                tok = t * (QT // P) + tt
                ps = mm_psum.tile([P, 512], f32, tag="mmp", name=f"fp{tok}_{eh}")
                for cc in range(CC):
                    nc.tensor.matmul(
                        ps, lhsT=ots[t][:, cc, ts(tt, P)],
                        rhs=wf_sb[:, cc, ts(eh, 512)],
                        start=(cc == 0), stop=(cc == CC - 1),
                    )
                ysb = ysb_pool.tile([P, 512], f32, tag="ysb", name=f"ys{tok}_{eh}")
                nc.vector.tensor_copy(out=ysb, in_=ps)
                nc.sync.dma_start(y_r[tok, :, ts(eh, 512)], ysb)

            class PairState:
                """QK/exp products of one head pair, awaiting AV drain."""

                def __init__(self, t, j):
                    self.t, self.j = t, j
                    self.es = es_pool.tile([P, nkt_a, 2, QT], bf16, tag="es",
                                           name=f"es{t}_{j}")
                    self.avs = [
                        av_psum.tile([P, QT], f32, tag="avp",
                                     name=f"avp{t}_{j}_{jj}")
                        for jj in range(2)
                    ]
                    self.av_kc = 0
                    self.stg = None

                def av_step(self):
                    kc = self.av_kc
                    for jj in range(2):
                        nc.tensor.matmul(
                            self.avs[jj],
                            lhsT=vaug[:, kc, 2 * self.j + jj, :],
                            rhs=self.es[:, kc, jj, :],
                            start=(kc == 0), stop=(kc == nkt_a - 1),
                        )
                    self.av_kc += 1

                def av_drain(self, upto):
                    while self.av_kc < upto:
                        self.av_step()

                def stage(self):
                    """Copy AV psums to SBUF (on ACT, which has slack) so the
                    PSUM slots free ~1.5us after the AV drain instead of after
                    the slow normalize DMA chain."""
                    t, j = self.t, self.j
                    self.stg = [
                        rc_pool.tile([P, QT], f32, tag="stg", bufs=6,
                                     name=f"sg{t}{j}{jj}")
                        for jj in range(2)
                    ]
                    nc.scalar.copy(
                        out=self.stg[0][0:DH + 1, :], in_=self.avs[0][0:DH + 1, :])
                    nc.scalar.copy(out=self.stg[1], in_=self.avs[1])

            def normalize_t(t, p0, p1):
                """Batched softmax normalization for q-tile t (both pairs).

                Denominator rows live at staged partitions 64 (even head) and
                0 (odd head).  One DVE reciprocal for all four rows, then a
                DRAM round-trip to partition-broadcast (only DRAM APs may have
                stride-0 partition dims)."""
                rall = rc_pool.tile([4, QT], f32, tag="rall", name=f"ra{t}")
                rr = rc_pool.tile([4, QT], f32, tag="rr", name=f"rr{t}")
                for i, (p, jj, row) in enumerate(
                        ((p0, 0, DH), (p0, 1, 0), (p1, 0, DH), (p1, 1, 0))):
                    nc.sync.dma_start(
                        rall[i:i + 1, :], p.stg[jj][row:row + 1, :])
                nc.vector.reciprocal(rr[0:4, :], rall[0:4, :])
                den_d = dram_pool.tile([4, QT], f32, tag="dend", name=f"dd{t}")
                nc.sync.dma_start(den_d, rr[0:4, :])
                for j, p in ((0, p0), (1, p1)):
                    rcb = rc_pool.tile([P, QT], f32, tag="rcb", name=f"rb{t}{j}")
                    nc.sync.dma_start(
                        rcb[0:DH, :],
                        den_d[2 * j, None, :].to_broadcast((DH, QT)))
                    nc.sync.dma_start(
                        rcb[DH:P, :],
                        den_d[2 * j + 1, None, :].to_broadcast((DH, QT)))
                    nc.vector.tensor_tensor(
                        out=ots[t][0:DH, j, :], in0=p.stg[0][0:DH, :],
                        in1=rcb[0:DH, :], op=MUL,
                    )
                    nc.vector.tensor_tensor(
                        out=ots[t][DH:P, j, :], in0=p.stg[1][DH:P, :],
                        in1=rcb[DH:P, :], op=MUL,
                    )

            def emit_pair(t, j, units, drain=None, self_av=False):
                """QK+exp loop for pair (t, j), interleaving `units` and the
                AV drain of a previous pair (and optionally its own)."""
                st = PairState(t, j)
                nu = len(units)
                ei = 0
                for kc in range(nkt_a):
                    stp = st_psum.tile([P, 2, QT], f32, tag="stp",
                                       name=f"st{t}_{j}_{kc}")
                    nc.tensor.matmul(
                        stp[:, 0, :],
                        lhsT=kt_sb[0:DH, j, ts(kc, P)],
                        rhs=qt_sb[0:DH, j, ts(t, QT)],
                        start=True, stop=True,
                    )
                    nc.tensor.matmul(
                        stp[:, 1, :],
                        lhsT=kt_sb[DH:P, j, ts(kc, P)],
                        rhs=qt_sb[DH:P, j, ts(t, QT)],
                        start=True, stop=True,
                    )
                    nc.scalar.activation(
                        out=st.es[:, kc, :, :], in_=stp[:, :, :],
                        func=Exp, scale=1.0 / DH,
                    )
                    target = (kc + 1) * nu // nkt_a
                    while ei < target:
                        units[ei]()
                        ei += 1
                    if drain is not None:
                        drain.av_drain(kc + 1)
                if drain is not None:
                    drain.av_drain(nkt_a)
                    drain.stage()
                if self_av:
                    st.av_drain(nkt_a)
                    st.stage()
                return st

            # ---- lead-in: just enough K/Q projection for the first pair ----
            emit_kq_proj(xk_r, wk_sb, bk_sb, kt_sb, 0, 0)
            emit_kq_proj(xq_r, wq_sb, bq_sb, qt_sb, 0, 0)
            nc.vector.memset(vaug, 0.0)
            nc.vector.tensor_copy(
                out=vaug_v[:, 0, :, :, DH],
                in_=m01_sb[:, :, None].to_broadcast((P, nkt_a, 2)),
            )
            nc.vector.tensor_copy(
                out=vaug_v[:, 1, :, :, 0],
                in_=m01_sb[:, :, None].to_broadcast((P, nkt_a, 2)),
            )

            ots = {
                t: ot_pool.tile([P, CC, QT], bf16, tag="ot", name=f"ot{t}")
                for t in range(NQT)
            }

            # remaining projection blocks as interleavable units
            k_units = [
                (lambda tt=tt, cc=cc: emit_kq_proj(xk_r, wk_sb, bk_sb, kt_sb, tt, cc))
                for cc in range(CC) for tt in range(KTILES) if not (tt == 0 and cc == 0)
            ]
            q0c1 = [lambda: emit_kq_proj(xq_r, wq_sb, bq_sb, qt_sb, 0, 1)]
            v_units = [
                (lambda tt=tt: emit_vproj_chunk(tt)) for tt in range(nkt_a)
            ]

            def qproj_units(t):
                return [
                    (lambda cc=cc, tn=t: emit_kq_proj(
                        xq_r, wq_sb, bq_sb, qt_sb, tn, cc))
                    for cc in range(CC)
                ]

            def f_units(t):
                return [
                    (lambda tt=tt, eh=eh, tp=t: emit_f_unit(tp, tt, eh))
                    for tt in range(QT // P) for eh in range(2)
                ]

            # Unit placement: ot(t-1) is complete only at the END of pair
            # (t, 0) (which drains pair (t-1, 1)), so f(t-1) units go in pair
            # (t, 1).  Qproj(t+1) must precede pair (t+1, 0): put it in (t, 0).
            prev = None
            pairs = {}
            for t in range(NQT):
                if t == 0:
                    u0 = k_units + q0c1 + qproj_units(1)
                    u1 = v_units
                else:
                    u0 = qproj_units(t + 1) if t < NQT - 1 else []
                    u1 = f_units(t - 1)
                p0 = emit_pair(t, 0, u0, drain=prev)
                if t >= 1:
                    normalize_t(t - 1, pairs[t - 1], prev)
                p1 = emit_pair(t, 1, u1, drain=p0,
                               self_av=(t == NQT - 1))
                pairs[t] = p0
                prev = p1
            # tail: normalize the last q-tile, then its output projection
            normalize_t(NQT - 1, pairs[NQT - 1], prev)
            for tt in range(QT // P):
                for eh in range(2):
                    emit_f_unit(NQT - 1, tt, eh)

    nc.compile()
    return nc


def _get_nc(nkt_a):
    if nkt_a not in _CACHE:
        _CACHE[nkt_a] = _build(nkt_a)
    return _CACHE[nkt_a]


def kernel(**inputs):
    global LAST_RESULTS
    query = np.asarray(inputs["query"], np.float32)
    key = np.asarray(inputs["key"], np.float32)
    value = np.asarray(inputs["value"], np.float32)
    pad_mask = np.asarray(inputs["pad_mask"])
    training = int(np.asarray(inputs["training_status"]))
    Wq = np.asarray(inputs["Wq"], np.float32)
    Wk = np.asarray(inputs["Wk"], np.float32)
    Wv = np.asarray(inputs["Wv"], np.float32)
    Wf = np.asarray(inputs["Wf"], np.float32)
    bq = np.asarray(inputs["bq"], np.float32)
    bk = np.asarray(inputs["bk"], np.float32)
    bv = np.asarray(inputs["bv"], np.float32)
    bf = np.asarray(inputs["bf"], np.float32)

    # Per-batch key permutation: unmasked keys first.  Attention is
    # permutation-invariant over keys, and fully-masked key chunks contribute
    # exactly zero (mask is folded into V and the denominator column), so the
    # kernel only needs ceil(max_unmasked / 128) key chunks.
    m01_full = {}
    perms = {}
    n_act = 1
    for b in range(B):
        if training:
            m = (pad_mask[b, 0, 0, :] != 0).astype(np.float32)
        else:
            m = np.ones(S, np.float32)
        perm = np.argsort(-m, kind="stable")
        m01_full[b] = m[perm]
        perms[b] = perm
        n_act = max(n_act, int(np.ceil(m.sum() / P)))
    nkt_a = min(NKT, max(2, n_act))
    ktok = ((nkt_a + 3) // 4) * QT

    nc = _get_nc(nkt_a)

    def prep_kv(x, b):
        xp = x[b][perms[b]]  # [S, DIM] permuted
        out = np.zeros((ktok, DIM), np.float32)
        out[: min(ktok, S)] = xp[:ktok]
        return np.ascontiguousarray(out.T).astype(BF16)

    xT = {}
    for b in range(B):
        xT[("q", b)] = np.ascontiguousarray(query[b].T).astype(BF16)
        xT[("k", b)] = prep_kv(key, b)
        xT[("v", b)] = prep_kv(value, b)
        m = np.zeros(nkt_a * P, np.float32)
        n = min(nkt_a * P, S)
        m[:n] = m01_full[b][:n]
        m01_full[b] = m

    in_maps = []
    for c in range(NCORES):
        b, g = divmod(c, HPC)
        cs = slice(g * CSL, (g + 1) * CSL)
        in_maps.append({
            "xq": xT[("q", b)],
            "xk": xT[("k", b)],
            "xv": xT[("v", b)],
            "wq": np.ascontiguousarray(Wq[:, cs]).astype(BF16),
            "wk": np.ascontiguousarray(Wk[:, cs]).astype(BF16),
            "wv": np.ascontiguousarray(Wv[:, cs]).astype(BF16),
            "wf": np.ascontiguousarray(Wf[cs, :]).astype(BF16),
            "bq": np.ascontiguousarray(bq[cs]),
            "bk": np.ascontiguousarray(bk[cs]),
            "bv": np.ascontiguousarray(bv[cs]),
            "m01": m01_full[b],
        })

    from concourse.bass_utils import run_bass_kernel_spmd

    res = run_bass_kernel_spmd(nc, in_maps, core_ids=list(range(NCORES)))
    LAST_RESULTS = res

    out = np.zeros((B, S, DIM), np.float32)
    for c in range(NCORES):
        b = c // HPC
        out[b] += res.results[c]["y"]
    out += bf[None, None, :]
    return out
